# revision 14
# baseline (speedup 1.0000x reference)
"""Bahdanau-attention scoring kernel for Trainium2 (8 NeuronCores, SPMD).

Computes softmax_s( v . tanh(hidden @ Wh^T + enc @ We^T + b) ) for
hidden [32,1024], enc [32,2048,1024]  ->  out [32,2048].

Sharding: data-parallel over batch (4 rows / core). Weights replicated.
Per core: energy in [o_part, s_free] layout via fp16 matmuls (We^T
stationary, enc^T moving), tanh fused with the host-precomputed bias
q = hidden@Wh^T + b on ScalarE.  v-dot: packed col-group matmuls ->
partials on partitions {0,32,64,96}, one DVE copy, one mask-matmul that
both reduces the partials and routes chunk c's scores to partition 32c.
Softmax without max subtraction (scores are ~ +-35, exp safe in fp32):
exp per chunk straight from PSUM into a [97,512] prob tile (row 32c =
chunk c) with accumulated sums; per batch one outer-mask matmul sums +
broadcasts the 4 partials, reciprocal + per-partition scale on DVE,
strided DMA out.  The v-dot/mask/exp for chunk i are deferred into
chunk i+1's instruction stream so the PE never waits on a tanh.
DMAs ride two HW queues (sync: enc, out; scalar: weights + small).
"""

from contextlib import ExitStack

import numpy as np

import concourse.bacc as bacc
import concourse.bass_isa as bass_isa
import concourse.library_config as library_config
import concourse.mybir as mybir
import concourse.tile as tile
from concourse.bass_utils import run_bass_kernel_spmd

HID = 1024
BATCH = 32
SRC = 2048
NCORES = 8
BLOC = BATCH // NCORES  # 4 batch rows per core
KT = HID // 128  # 8 k-tiles over the contraction dim
MT = HID // 128  # 8 m-tiles over the output-feature dim
NCHUNK = 512  # matmul moving free dim / psum bank width (fp32 out)
SCHUNKS = SRC // NCHUNK  # 4 s-chunks per batch row
NCHUNKS = BLOC * SCHUNKS  # 16 chunks per core

F32 = mybir.dt.float32
F32R = mybir.dt.float32r
F16 = mybir.dt.float16

_compiled = {}
_last_results = None


def _build_kernel(ctx: ExitStack, tc: tile.TileContext, aps: dict):
    nc = tc.nc
    enc_d = aps["enc_t"]  # [BLOC, KT, 128, SRC] (b, k, p, s) fp16
    we_d = aps["w_et"]  # [MT, 128, KT * 128]  (m, p, k*o') fp16
    q_d = aps["q_t"]  # [128, MT * BLOC] f32 (Wh@hid^T + b, host)
    v_d = aps["v_t"]  # [128, MT] fp16
    out_d = aps["out"]  # [1, BLOC * SRC] fp32

    w_pool = ctx.enter_context(tc.tile_pool(name="w", bufs=1))
    small_pool = ctx.enter_context(tc.tile_pool(name="small", bufs=1))
    enc_pool = ctx.enter_context(tc.tile_pool(name="enc", bufs=4))
    tanh_pool = ctx.enter_context(tc.tile_pool(name="tanh", bufs=16))
    prob_pool = ctx.enter_context(tc.tile_pool(name="prob", bufs=2))
    stat_pool = ctx.enter_context(tc.tile_pool(name="stat", bufs=4))
    acc_pool = ctx.enter_context(tc.tile_pool(name="acc", bufs=2))
    prod_pool = ctx.enter_context(tc.tile_pool(name="prod", bufs=2))
    red_pool = ctx.enter_context(tc.tile_pool(name="red", bufs=2))
    psum_e = ctx.enter_context(tc.tile_pool(name="psum_e", bufs=6, space="PSUM"))
    psum_s = ctx.enter_context(tc.tile_pool(name="psum_s", bufs=2, space="PSUM"))

    nc.gpsimd.load_library(library_config.attn)

    # force the ACT table load (tanh/exp set) while DMAs are in flight
    warm = small_pool.tile([1, 2], F32)
    nc.vector.memset(warm[:], 0.0)
    nc.scalar.activation(
        warm[0:1, 1:2], warm[0:1, 0:1], mybir.ActivationFunctionType.Tanh
    )

    # scalar-queue DMAs: just the small resident tensors; the big enc0 +
    # We blocks go on the sync queue so enc0 gets full HBM bandwidth
    q_sb = small_pool.tile([128, MT * BLOC], F32)
    nc.scalar.dma_start(out=q_sb[:], in_=q_d[:])
    v_sb = small_pool.tile([128, MT], F16)
    nc.scalar.dma_start(out=v_sb[:], in_=v_d[:])
    v32_sb = small_pool.tile([128, MT], F32)
    nc.scalar.dma_start(out=v32_sb[:], in_=aps["v32_t"][:])

    # sync-queue: first enc chunk, then the We blocks (per-m so m0 can
    # start while later blocks stream), then per-chunk enc in the loop
    enc0_sb = enc_pool.tile([128, KT, NCHUNK], F16, tag="enc", name="enc0_sb")
    nc.sync.dma_start(
        out=enc0_sb[:],
        in_=enc_d[0].rearrange("k p s -> p k s")[:, :, 0:NCHUNK],
    )
    w_sb = w_pool.tile([128, MT, KT * 128], F16)
    for m in range(MT):
        nc.sync.dma_start(out=w_sb[:, m, :], in_=we_d[m])

    # warm the PE clock (HAM) with dummy matmuls on scratch while the
    # enc0/We DMAs are in flight, so chunk 0 runs at 2.4 GHz
    scr = small_pool.tile([128, NCHUNK], F16)
    nc.vector.memset(scr[:], 0.0)
    for _ in range(9):
        wp = psum_s.tile([128, NCHUNK], F32, tag="sc", name="warmmm")
        nc.tensor.matmul(
            wp[:], lhsT=scr[:, 0:128], rhs=scr[:], start=True, stop=True
        )

    def energy_m(enc_sb, m):
        ep = psum_e.tile([128, NCHUNK], F32, tag="ep", name="ep")
        for k in range(KT):
            nc.tensor.matmul(
                ep[:],
                lhsT=w_sb[:, m, k * 128 : (k + 1) * 128],
                rhs=enc_sb[:, k, :],
                start=(k == 0),
                stop=(k == KT - 1),
            )
        return ep

    def tanh_m(ep, b, m):
        th = tanh_pool.tile([128, NCHUNK], F16, name="th")
        nc.scalar.activation(
            th[:],
            ep[:],
            mybir.ActivationFunctionType.Tanh,
            bias=q_sb[:, m * BLOC + b : m * BLOC + b + 1],
            scale=1.0,
        )
        return th

    def vdot_dve(th_tiles):
        # v-dot off the PE: fused (th*v + acc) per m on DVE in fp32, then
        # one gpsimd partition all-reduce; row 0 holds the scores.
        acc = acc_pool.tile([128, NCHUNK], F32, name="acc")
        nc.vector.tensor_scalar_mul(acc[:], th_tiles[0][:], v32_sb[:, 0:1])
        for m in range(1, MT):
            nxt = acc_pool.tile([128, NCHUNK], F32, name="acc")
            nc.vector.scalar_tensor_tensor(
                nxt[:],
                th_tiles[m][:],
                v32_sb[:, m : m + 1],
                acc[:],
                op0=mybir.AluOpType.mult,
                op1=mybir.AluOpType.add,
            )
            acc = nxt
        red = red_pool.tile([128, NCHUNK], F32, name="red")
        nc.gpsimd.partition_all_reduce(
            red[:], acc[:], channels=128, reduce_op=bass_isa.ReduceOp.add
        )
        return red

    def exp_chunk(sc, prob_sb, esum, s):
        nc.scalar.activation(
            prob_sb[0:1, s * NCHUNK : (s + 1) * NCHUNK],
            sc[0:1, :],
            mybir.ActivationFunctionType.Exp,
            scale=1.0,
            accum_out=esum[0:1, s : s + 1],
        )

    def start_batch():
        prob_sb = prob_pool.tile([1, SRC], F32)
        esum = stat_pool.tile([1, SCHUNKS], F32, name="esum")
        return (prob_sb, esum)

    def finish_batch(b, prob_sb, esum):
        tot = stat_pool.tile([1, 1], F32, name="tot")
        nc.vector.tensor_reduce(
            tot[:], esum[:], axis=mybir.AxisListType.X, op=mybir.AluOpType.add
        )
        rcp = stat_pool.tile([1, 1], F32, name="rcp")
        nc.vector.reciprocal(rcp[:], tot[:])
        nc.vector.tensor_scalar_mul(prob_sb[:], prob_sb[:], rcp[0:1, 0:1])
        nc.sync.dma_start(out=out_d[0:1, b * SRC : (b + 1) * SRC], in_=prob_sb[:])

    # --- main loop: 16 chunks of 512 s-values ---------------------------
    # chunk i's v-dot runs after chunk i+1's first energy group, its
    # mask-mm after the second, its exp after the third tanh, and a
    # batch's normalization two chunks after its last chunk — so no PE
    # instruction ever waits on ScalarE/DVE latency.
    pend_th = None  # th tiles of chunk i-1
    pend_sc = None  # (vs|sp, prob, esum, s, b) awaiting exp
    pend_fin = None  # batch index awaiting normalization
    batch_ctx = {}
    for i in range(NCHUNKS):
        b, s = divmod(i, SCHUNKS)
        if s == 0:
            batch_ctx[b] = start_batch()
        prob_sb, esum = batch_ctx[b]
        if i == 0:
            enc_sb = enc0_sb
        else:
            enc_sb = enc_pool.tile([128, KT, NCHUNK], F16, tag="enc")
            nc.sync.dma_start(
                out=enc_sb[:],
                in_=enc_d[b].rearrange("k p s -> p k s")[
                    :, :, s * NCHUNK : (s + 1) * NCHUNK
                ],
            )
        last = i == NCHUNKS - 1
        th_tiles = []
        sp = None
        for m in range(MT):
            ep = energy_m(enc_sb, m)
            if m == 1 and pend_th is not None:
                # deferred v-dot of chunk i-1 (its tanhs are all done)
                pred = vdot_dve(pend_th)
            if m == 2 and pend_fin is not None:
                finish_batch(pend_fin, *batch_ctx[pend_fin])
                pend_fin = None
            if last and m >= 2:
                # tail chunk: plain accumulating v-dot, interleaved with
                # the energy groups, straight into score row 96
                if m == 2:
                    sp = psum_s.tile([128, NCHUNK], F32, tag="sc", name="sp")
                nc.tensor.matmul(
                    sp[0:1, :],
                    lhsT=v_sb[:, m - 2 : m - 1],
                    rhs=th_tiles[m - 2][:],
                    start=(m == 2),
                    stop=False,
                )
            th_tiles.append(tanh_m(ep, b, m))
        if pend_th is not None:
            # exp of chunk i-1 after this chunk's tanhs (the gpsimd
            # reduce finishes mid-iteration; ScalarE picks it up late)
            exp_chunk(pred, pend_sc[1], pend_sc[2], pend_sc[3])
            if pend_sc[3] == SCHUNKS - 1:
                pend_fin = pend_sc[4]
            pend_th = None
        if last:
            for m in range(MT - 2, MT):
                nc.tensor.matmul(
                    sp[0:1, :],
                    lhsT=v_sb[:, m : m + 1],
                    rhs=th_tiles[m][:],
                    start=False,
                    stop=(m == MT - 1),
                )
        else:
            pend_th = th_tiles
            pend_sc = (None, prob_sb, esum, s, b)
    # tail: exp + normalization for the last chunk/batch
    prob_sb, esum = batch_ctx[BLOC - 1]
    exp_chunk(sp, prob_sb, esum, SCHUNKS - 1)
    finish_batch(BLOC - 1, prob_sb, esum)


def build_nc():
    nc = bacc.Bacc("TRN2", target_bir_lowering=False, debug=False)
    aps = {
        "enc_t": nc.dram_tensor(
            "enc_t", [BLOC, KT, 128, SRC], F16, kind="ExternalInput"
        ).ap(),
        "w_et": nc.dram_tensor(
            "w_et", [MT, 128, KT * 128], F16, kind="ExternalInput"
        ).ap(),
        "q_t": nc.dram_tensor(
            "q_t", [128, MT * BLOC], F32, kind="ExternalInput"
        ).ap(),
        "v_t": nc.dram_tensor("v_t", [128, MT], F16, kind="ExternalInput").ap(),
        "v32_t": nc.dram_tensor(
            "v32_t", [128, MT], F32, kind="ExternalInput"
        ).ap(),
        "out": nc.dram_tensor(
            "out", [1, BLOC * SRC], F32, kind="ExternalOutput"
        ).ap(),
    }
    with tile.TileContext(nc, pool_alloc_mode="queue") as tc, ExitStack() as ctx:
        _build_kernel(ctx, tc, aps)
    nc.compile()
    return nc


def _prep_shared(hidden, attn_w, attn_b, v):
    w_e_t = np.ascontiguousarray(attn_w[:, HID:].T)  # [h, o]
    # [h, o] -> [kt, 128p, mt, 128o'] -> [mt, 128p, kt, 128o']
    w_et = np.ascontiguousarray(
        w_e_t.reshape(KT, 128, MT, 128)
        .transpose(2, 1, 0, 3)
        .reshape(MT, 128, KT * 128)
        .astype(np.float16)
    )
    v_t = np.ascontiguousarray(v.reshape(MT, 128).T.astype(np.float16))  # [128, mt]
    v32_t = np.ascontiguousarray(v.reshape(MT, 128).T.astype(np.float32))
    # q[o, b] = Wh @ hidden^T + b, fp32 on host (tiny GEMM)
    q_all = hidden @ attn_w[:, :HID].T + attn_b  # [BATCH, HID]
    q_cores = []
    for c in range(NCORES):
        qc = q_all[c * BLOC : (c + 1) * BLOC].T  # [HID, BLOC]
        q_cores.append(
            np.ascontiguousarray(
                qc.reshape(MT, 128, BLOC)
                .transpose(1, 0, 2)
                .reshape(128, MT * BLOC)
                .astype(np.float32)
            )
        )
    return w_et, v_t, v32_t, q_cores


def kernel(hidden, encoder_outputs, attn_w, attn_b, v):
    global _last_results
    hidden = np.asarray(hidden, dtype=np.float32)
    encoder_outputs = np.asarray(encoder_outputs, dtype=np.float32)
    attn_w = np.asarray(attn_w, dtype=np.float32)
    attn_b = np.asarray(attn_b, dtype=np.float32)
    v = np.asarray(v, dtype=np.float32)

    if "nc" not in _compiled:
        _compiled["nc"] = build_nc()
    nc = _compiled["nc"]

    w_et, v_t, v32_t, q_cores = _prep_shared(hidden, attn_w, attn_b, v)
    in_maps = []
    for c in range(NCORES):
        enc_c = encoder_outputs[c * BLOC : (c + 1) * BLOC]  # [bloc, s, h]
        # [bloc, s, h] -> [bloc, h, s] fp16 -> [bloc, kt, 128, s]
        enc_t = (
            np.ascontiguousarray(enc_c.transpose(0, 2, 1))
            .astype(np.float16)
            .reshape(BLOC, KT, 128, SRC)
        )
        in_maps.append(
            {
                "enc_t": enc_t,
                "w_et": w_et,
                "q_t": q_cores[c],
                "v_t": v_t,
                "v32_t": v32_t,
            }
        )

    res = run_bass_kernel_spmd(nc, in_maps, list(range(NCORES)))
    _last_results = res
    out = np.concatenate(
        [res.results[c]["out"].reshape(BLOC, SRC) for c in range(NCORES)], axis=0
    )
    return out.astype(np.float32)


# revision 15
# speedup vs baseline: 1.0083x; 1.0083x over previous
"""Bahdanau-attention scoring kernel for Trainium2 (8 NeuronCores, SPMD).

Computes softmax_s( v . tanh(hidden @ Wh^T + enc @ We^T + b) ) for
hidden [32,1024], enc [32,2048,1024]  ->  out [32,2048].

Sharding: data-parallel over batch (4 rows / core). Weights replicated.
Per core: energy in [o_part, s_free] layout via fp16 matmuls (We^T
stationary, enc^T moving), tanh fused with the host-precomputed bias
q = hidden@Wh^T + b on ScalarE.  v-dot: packed col-group matmuls ->
partials on partitions {0,32,64,96}, one DVE copy, one mask-matmul that
both reduces the partials and routes chunk c's scores to partition 32c.
Softmax without max subtraction (scores are ~ +-35, exp safe in fp32):
exp per chunk straight from PSUM into a [97,512] prob tile (row 32c =
chunk c) with accumulated sums; per batch one outer-mask matmul sums +
broadcasts the 4 partials, reciprocal + per-partition scale on DVE,
strided DMA out.  The v-dot/mask/exp for chunk i are deferred into
chunk i+1's instruction stream so the PE never waits on a tanh.
DMAs ride two HW queues (sync: enc, out; scalar: weights + small).
"""

from contextlib import ExitStack

import numpy as np

import concourse.bacc as bacc
import concourse.bass_isa as bass_isa
import concourse.library_config as library_config
import concourse.mybir as mybir
import concourse.tile as tile
from concourse.bass_utils import run_bass_kernel_spmd

HID = 1024
BATCH = 32
SRC = 2048
NCORES = 8
BLOC = BATCH // NCORES  # 4 batch rows per core
KT = HID // 128  # 8 k-tiles over the contraction dim
MT = HID // 128  # 8 m-tiles over the output-feature dim
NCHUNK = 512  # matmul moving free dim / psum bank width (fp32 out)
SCHUNKS = SRC // NCHUNK  # 4 s-chunks per batch row
NCHUNKS = BLOC * SCHUNKS  # 16 chunks per core

F32 = mybir.dt.float32
F32R = mybir.dt.float32r
F16 = mybir.dt.float16

_compiled = {}
_last_results = None


def _build_kernel(ctx: ExitStack, tc: tile.TileContext, aps: dict):
    nc = tc.nc
    enc_d = aps["enc_t"]  # [BLOC, KT, 128, SRC] (b, k, p, s) fp16
    we_d = aps["w_et"]  # [MT, 128, KT * 128]  (m, p, k*o') fp16
    q_d = aps["q_t"]  # [128, MT * BLOC] f32 (Wh@hid^T + b, host)
    v_d = aps["v_t"]  # [128, MT] fp16
    out_d = aps["out"]  # [1, BLOC * SRC] fp32

    w_pool = ctx.enter_context(tc.tile_pool(name="w", bufs=1))
    small_pool = ctx.enter_context(tc.tile_pool(name="small", bufs=1))
    enc_pool = ctx.enter_context(tc.tile_pool(name="enc", bufs=4))
    tanh_pool = ctx.enter_context(tc.tile_pool(name="tanh", bufs=16))
    prob_pool = ctx.enter_context(tc.tile_pool(name="prob", bufs=2))
    stat_pool = ctx.enter_context(tc.tile_pool(name="stat", bufs=4))
    acc_pool = ctx.enter_context(tc.tile_pool(name="acc", bufs=2))
    prod_pool = ctx.enter_context(tc.tile_pool(name="prod", bufs=2))
    red_pool = ctx.enter_context(tc.tile_pool(name="red", bufs=2))
    psum_e = ctx.enter_context(tc.tile_pool(name="psum_e", bufs=6, space="PSUM"))
    psum_s = ctx.enter_context(tc.tile_pool(name="psum_s", bufs=2, space="PSUM"))

    nc.gpsimd.load_library(library_config.attn)

    # force the ACT table load (tanh/exp set) while DMAs are in flight
    warm = small_pool.tile([1, 2], F32)
    nc.vector.memset(warm[:], 0.0)
    nc.scalar.activation(
        warm[0:1, 1:2], warm[0:1, 0:1], mybir.ActivationFunctionType.Tanh
    )

    # scalar-queue DMAs: just the small resident tensors; the big enc0 +
    # We blocks go on the sync queue so enc0 gets full HBM bandwidth
    q_sb = small_pool.tile([128, MT * BLOC], F32)
    nc.scalar.dma_start(out=q_sb[:], in_=q_d[:])
    v_sb = small_pool.tile([128, MT], F16)
    nc.scalar.dma_start(out=v_sb[:], in_=v_d[:])
    v32_sb = small_pool.tile([128, MT], F32)
    nc.scalar.dma_start(out=v32_sb[:], in_=aps["v32_t"][:])

    # sync-queue: first enc chunk, then the We blocks (per-m so m0 can
    # start while later blocks stream), then per-chunk enc in the loop
    enc0_sb = enc_pool.tile([128, KT, NCHUNK], F16, tag="enc", name="enc0_sb")
    nc.sync.dma_start(
        out=enc0_sb[:],
        in_=enc_d[0].rearrange("k p s -> p k s")[:, :, 0:NCHUNK],
    )
    w_sb = w_pool.tile([128, MT, KT * 128], F16)
    for m in range(MT):
        nc.sync.dma_start(out=w_sb[:, m, :], in_=we_d[m])

    # warm the PE clock (HAM) with dummy matmuls on scratch while the
    # enc0/We DMAs are in flight, so chunk 0 runs at 2.4 GHz
    scr = small_pool.tile([128, NCHUNK], F16)
    nc.vector.memset(scr[:], 0.0)
    for _ in range(18):
        wp = psum_s.tile([128, NCHUNK], F32, tag="sc", name="warmmm")
        nc.tensor.matmul(
            wp[:], lhsT=scr[:, 0:128], rhs=scr[:], start=True, stop=True
        )

    def energy_m(enc_sb, m):
        ep = psum_e.tile([128, NCHUNK], F32, tag="ep", name="ep")
        for k in range(KT):
            nc.tensor.matmul(
                ep[:],
                lhsT=w_sb[:, m, k * 128 : (k + 1) * 128],
                rhs=enc_sb[:, k, :],
                start=(k == 0),
                stop=(k == KT - 1),
            )
        return ep

    def tanh_m(ep, b, m):
        th = tanh_pool.tile([128, NCHUNK], F16, name="th")
        nc.scalar.activation(
            th[:],
            ep[:],
            mybir.ActivationFunctionType.Tanh,
            bias=q_sb[:, m * BLOC + b : m * BLOC + b + 1],
            scale=1.0,
        )
        return th

    def vdot_dve(th_tiles):
        # v-dot off the PE: fused (th*v + acc) per m on DVE in fp32, then
        # one gpsimd partition all-reduce; row 0 holds the scores.
        acc = acc_pool.tile([128, NCHUNK], F32, name="acc")
        nc.vector.tensor_scalar_mul(acc[:], th_tiles[0][:], v32_sb[:, 0:1])
        for m in range(1, MT):
            nxt = acc_pool.tile([128, NCHUNK], F32, name="acc")
            nc.vector.scalar_tensor_tensor(
                nxt[:],
                th_tiles[m][:],
                v32_sb[:, m : m + 1],
                acc[:],
                op0=mybir.AluOpType.mult,
                op1=mybir.AluOpType.add,
            )
            acc = nxt
        red = red_pool.tile([128, NCHUNK], F32, name="red")
        nc.gpsimd.partition_all_reduce(
            red[:], acc[:], channels=128, reduce_op=bass_isa.ReduceOp.add
        )
        return red

    def exp_chunk(sc, prob_sb, esum, s):
        nc.scalar.activation(
            prob_sb[0:1, s * NCHUNK : (s + 1) * NCHUNK],
            sc[0:1, :],
            mybir.ActivationFunctionType.Exp,
            scale=1.0,
            accum_out=esum[0:1, s : s + 1],
        )

    def start_batch():
        prob_sb = prob_pool.tile([1, SRC], F32)
        esum = stat_pool.tile([1, SCHUNKS], F32, name="esum")
        return (prob_sb, esum)

    def finish_batch(b, prob_sb, esum):
        tot = stat_pool.tile([1, 1], F32, name="tot")
        nc.vector.tensor_reduce(
            tot[:], esum[:], axis=mybir.AxisListType.X, op=mybir.AluOpType.add
        )
        rcp = stat_pool.tile([1, 1], F32, name="rcp")
        nc.vector.reciprocal(rcp[:], tot[:])
        nc.vector.tensor_scalar_mul(prob_sb[:], prob_sb[:], rcp[0:1, 0:1])
        nc.sync.dma_start(out=out_d[0:1, b * SRC : (b + 1) * SRC], in_=prob_sb[:])

    # --- main loop: 16 chunks of 512 s-values ---------------------------
    # chunk i's v-dot runs after chunk i+1's first energy group, its
    # mask-mm after the second, its exp after the third tanh, and a
    # batch's normalization two chunks after its last chunk — so no PE
    # instruction ever waits on ScalarE/DVE latency.
    pend_th = None  # th tiles of chunk i-1
    pend_sc = None  # (vs|sp, prob, esum, s, b) awaiting exp
    pend_fin = None  # batch index awaiting normalization
    batch_ctx = {}
    for i in range(NCHUNKS):
        b, s = divmod(i, SCHUNKS)
        if s == 0:
            batch_ctx[b] = start_batch()
        prob_sb, esum = batch_ctx[b]
        if i == 0:
            enc_sb = enc0_sb
        else:
            enc_sb = enc_pool.tile([128, KT, NCHUNK], F16, tag="enc")
            nc.sync.dma_start(
                out=enc_sb[:],
                in_=enc_d[b].rearrange("k p s -> p k s")[
                    :, :, s * NCHUNK : (s + 1) * NCHUNK
                ],
            )
        last = i == NCHUNKS - 1
        th_tiles = []
        sp = None
        for m in range(MT):
            ep = energy_m(enc_sb, m)
            if m == 1 and pend_th is not None:
                # deferred v-dot of chunk i-1 (its tanhs are all done)
                pred = vdot_dve(pend_th)
            if m == 2 and pend_fin is not None:
                finish_batch(pend_fin, *batch_ctx[pend_fin])
                pend_fin = None
            if last and m >= 2:
                # tail chunk: plain accumulating v-dot, interleaved with
                # the energy groups, straight into score row 96
                if m == 2:
                    sp = psum_s.tile([128, NCHUNK], F32, tag="sc", name="sp")
                nc.tensor.matmul(
                    sp[0:1, :],
                    lhsT=v_sb[:, m - 2 : m - 1],
                    rhs=th_tiles[m - 2][:],
                    start=(m == 2),
                    stop=False,
                )
            th_tiles.append(tanh_m(ep, b, m))
        if pend_th is not None:
            # exp of chunk i-1 after this chunk's tanhs (the gpsimd
            # reduce finishes mid-iteration; ScalarE picks it up late)
            exp_chunk(pred, pend_sc[1], pend_sc[2], pend_sc[3])
            if pend_sc[3] == SCHUNKS - 1:
                pend_fin = pend_sc[4]
            pend_th = None
        if last:
            for m in range(MT - 2, MT):
                nc.tensor.matmul(
                    sp[0:1, :],
                    lhsT=v_sb[:, m : m + 1],
                    rhs=th_tiles[m][:],
                    start=False,
                    stop=(m == MT - 1),
                )
        else:
            pend_th = th_tiles
            pend_sc = (None, prob_sb, esum, s, b)
    # tail: exp + normalization for the last chunk/batch
    prob_sb, esum = batch_ctx[BLOC - 1]
    exp_chunk(sp, prob_sb, esum, SCHUNKS - 1)
    finish_batch(BLOC - 1, prob_sb, esum)


def build_nc():
    nc = bacc.Bacc("TRN2", target_bir_lowering=False, debug=False)
    aps = {
        "enc_t": nc.dram_tensor(
            "enc_t", [BLOC, KT, 128, SRC], F16, kind="ExternalInput"
        ).ap(),
        "w_et": nc.dram_tensor(
            "w_et", [MT, 128, KT * 128], F16, kind="ExternalInput"
        ).ap(),
        "q_t": nc.dram_tensor(
            "q_t", [128, MT * BLOC], F32, kind="ExternalInput"
        ).ap(),
        "v_t": nc.dram_tensor("v_t", [128, MT], F16, kind="ExternalInput").ap(),
        "v32_t": nc.dram_tensor(
            "v32_t", [128, MT], F32, kind="ExternalInput"
        ).ap(),
        "out": nc.dram_tensor(
            "out", [1, BLOC * SRC], F32, kind="ExternalOutput"
        ).ap(),
    }
    with tile.TileContext(nc, pool_alloc_mode="queue") as tc, ExitStack() as ctx:
        _build_kernel(ctx, tc, aps)
    nc.compile()
    return nc


def _prep_shared(hidden, attn_w, attn_b, v):
    w_e_t = np.ascontiguousarray(attn_w[:, HID:].T)  # [h, o]
    # [h, o] -> [kt, 128p, mt, 128o'] -> [mt, 128p, kt, 128o']
    w_et = np.ascontiguousarray(
        w_e_t.reshape(KT, 128, MT, 128)
        .transpose(2, 1, 0, 3)
        .reshape(MT, 128, KT * 128)
        .astype(np.float16)
    )
    v_t = np.ascontiguousarray(v.reshape(MT, 128).T.astype(np.float16))  # [128, mt]
    v32_t = np.ascontiguousarray(v.reshape(MT, 128).T.astype(np.float32))
    # q[o, b] = Wh @ hidden^T + b, fp32 on host (tiny GEMM)
    q_all = hidden @ attn_w[:, :HID].T + attn_b  # [BATCH, HID]
    q_cores = []
    for c in range(NCORES):
        qc = q_all[c * BLOC : (c + 1) * BLOC].T  # [HID, BLOC]
        q_cores.append(
            np.ascontiguousarray(
                qc.reshape(MT, 128, BLOC)
                .transpose(1, 0, 2)
                .reshape(128, MT * BLOC)
                .astype(np.float32)
            )
        )
    return w_et, v_t, v32_t, q_cores


def kernel(hidden, encoder_outputs, attn_w, attn_b, v):
    global _last_results
    hidden = np.asarray(hidden, dtype=np.float32)
    encoder_outputs = np.asarray(encoder_outputs, dtype=np.float32)
    attn_w = np.asarray(attn_w, dtype=np.float32)
    attn_b = np.asarray(attn_b, dtype=np.float32)
    v = np.asarray(v, dtype=np.float32)

    if "nc" not in _compiled:
        _compiled["nc"] = build_nc()
    nc = _compiled["nc"]

    w_et, v_t, v32_t, q_cores = _prep_shared(hidden, attn_w, attn_b, v)
    in_maps = []
    for c in range(NCORES):
        enc_c = encoder_outputs[c * BLOC : (c + 1) * BLOC]  # [bloc, s, h]
        # [bloc, s, h] -> [bloc, h, s] fp16 -> [bloc, kt, 128, s]
        enc_t = (
            np.ascontiguousarray(enc_c.transpose(0, 2, 1))
            .astype(np.float16)
            .reshape(BLOC, KT, 128, SRC)
        )
        in_maps.append(
            {
                "enc_t": enc_t,
                "w_et": w_et,
                "q_t": q_cores[c],
                "v_t": v_t,
                "v32_t": v32_t,
            }
        )

    res = run_bass_kernel_spmd(nc, in_maps, list(range(NCORES)))
    _last_results = res
    out = np.concatenate(
        [res.results[c]["out"].reshape(BLOC, SRC) for c in range(NCORES)], axis=0
    )
    return out.astype(np.float32)
